# revision 26
# baseline (speedup 1.0000x reference)
"""Multi-head attention (B=2, S=4096, D=768, H=12, hd=64) on 8 trn2 NeuronCores.

Sharding: core c -> batch b = c//4, heads [3*(c%4), 3*(c%4)+3)  (batch- and
head-parallel; no device collectives).  Each core computes the partial
output  sum_h softmax((x Wq_h + bq_h)(x Wk_h + bk_h)^T / 8) (x Wv_h) Wo_h
for its 3 heads as a full [S, 768] bf16 tensor; the host sums the 4 partials
per batch and adds the bias terms (bo + bv @ Wo, since softmax rows sum to 1).

Key device-side structure (v3):
  - scores run as ROW-TILED K=64 matmul pairs: heads 0/1 live on SBUF
    partition halves 0:64 / 64:128 of merged qt01/kt01 tiles, so two score
    matmuls (tile_position (0,0)/(64,0), auto-derived from AP base
    partitions) stream CONCURRENTLY through the PE array (trace-verified
    dstart ~3ns) — 2x score throughput vs the K=128 zero-padded form.
    Head 2 pairs with itself: k-blocks 0:16 on rows 0:64, 16:32 on rows
    64:128 (q2 duplicated onto both halves via partition-shift DMA).
  - exp split ~6/11 ACT (true exp -> fp8e4m3), 5/11 DVE via a Schraudolph
    bit-trick: int8(S/ln2 + 56) IS the fp8e4m3 bit pattern of exp(S/8)
    (rel_l2 ~1e-2 vs 8.4e-3 all-ACT, sim-verified).
  - P@V via fp8 DoubleRow; P tiles [128, 2, 2, 512] (pair-pos x head x q):
    one exp fills both heads' planes, each DR call reads one head's plane
    pair at stride 1024B.
  - prologue: fine-grained x DMA + k-proj interleaved with the first
    q-chunk's scores so the exp stream starts ~8us in; fc_out copies and
    V-quantize on ACT (Copy shares the exp table set); bf16 output.
  - PSUM banks: 3x[128,2,512] score conveyor (6; prologue proj tiles ride
    in half-used conveyor slots) + 1 fin + 1 P@V accumulator (heads 0/1
    accumulate in two sequential passes) = 8.
"""

import numpy as np
from contextlib import ExitStack

import concourse.bass as bass
import concourse.bacc as bacc
import concourse.mybir as mybir
from concourse import tile

BF16 = mybir.dt.bfloat16
F32 = mybir.dt.float32
F8 = mybir.dt.float8e4
I8 = mybir.dt.int8
AF = mybir.ActivationFunctionType
DR = mybir.MatmulPerfMode.DoubleRow

D_MODEL = 768
N_HEADS = 12
HD = 64
N_CORES = 8
NH_LOC = 3           # heads per core
DC = D_MODEL // 128  # 6 chunks of d_model
CHUNK = 512          # q columns processed per score chunk

# Schraudolph int8 exp: int8(round(S * 1/ln2 + 56)) == fp8e4m3 bits of
# exp(S/8) (3 mantissa bits, bias 7; valid for |S| < ~49).
SCHRAUD_A = float(1.0 / np.log(2.0))
SCHRAUD_B = 56.0
# exp-call engine assignment: DVE takes these slots of every PAT_N calls
DVE_PAT = (1, 3, 5, 7, 9, 11, 13, 15)
PAT_N = 17


def build(nc, S, level=3):
    SB = S // 128     # seq blocks of 128
    NCH = S // CHUNK  # q chunks
    KB = S // 128     # k blocks of 128
    HKB = KB // 2     # k blocks per partition half (head 2)

    xT_d = nc.declare_dram_parameter("xT", [DC, 128, S], BF16, isOutput=False)
    wqk_d = nc.declare_dram_parameter("wqk", [3, DC, 128, 128], BF16, isOutput=False)
    bqk_d = nc.declare_dram_parameter("bqk", [128, 3], F32, isOutput=False)
    wv_d = nc.declare_dram_parameter("wv", [DC, 128, NH_LOC * HD], BF16, isOutput=False)
    wo_d = nc.declare_dram_parameter("wo", [2, 128, D_MODEL], BF16, isOutput=False)
    out_d = nc.declare_dram_parameter("out", [S, D_MODEL], BF16, isOutput=True)

    with tile.TileContext(nc) as tc, ExitStack() as ctx:
        const = ctx.enter_context(tc.tile_pool(name="const", bufs=1))

        def ctile(name, shape, dt):
            return const.tile(shape, dt, tag=name, name=name)

        # --- constants / long-lived tensors -------------------------------
        XH = S // 2 if S >= 1024 else S   # xT column-half size
        xts = [ctile(f"xt{i}", [128, XH], BF16)
               for i in range(DC * (S // XH))]

        def xth(dcc, off, ln):
            t = xts[dcc * (S // XH) + off // XH]
            lo = off % XH
            return t[:, lo:lo + ln]
        wqks = [ctile(f"wqk{i}", [128, DC * 128], BF16) for i in range(3)]
        bqks = ctile("bqk", [128, 3], F32)
        wvs = [ctile(f"wv{i}", [128, NH_LOC * HD], BF16) for i in range(DC)]
        wos = [ctile(f"wo{i}", [128, D_MODEL], BF16) for i in range(2)]
        v1s = [ctile(f"v1_{h}", [128, KB, 128], F8) for h in range(NH_LOC)]
        warmt = ctile("warm", [128, 640], BF16)
        # merged q/k tiles: head 0 on rows 0:64, head 1 on rows 64:128
        qt01 = ctile("qt01", [128, S], BF16)
        kt01 = ctile("kt01", [128, S], BF16)
        # head 2: q duplicated on both halves; k blocks 0:HKB on rows 0:64,
        # blocks HKB:KB on rows 64:128 (block m at cols (m%HKB)*128)
        qt2d = ctile("qt2d", [128, S], BF16)
        kt2d = ctile("kt2d", [128, HKB * 128], BF16)
        # fin lhsT: heads 0 (rows 0:64) + 1 (rows 64:128) share atp;
        # at2: head 2 rows 0:64, rows 64:128 zero-padded
        atp = [ctile(f"atp{qc}", [128, CHUNK], BF16) for qc in range(NCH)]
        at2 = [ctile(f"at2_{qc}", [128, CHUNK], BF16) for qc in range(NCH)]

        pt_pool = ctx.enter_context(tc.tile_pool(name="pt", bufs=38))
        outst_pool = ctx.enter_context(tc.tile_pool(name="outst", bufs=2))
        small_pool = ctx.enter_context(tc.tile_pool(name="small", bufs=2))
        rb_pool = ctx.enter_context(tc.tile_pool(name="rb", bufs=2))
        dram_pool = ctx.enter_context(tc.tile_pool(name="drs", bufs=3, space="DRAM"))
        # PSUM (8 banks): 3x[128,2,512] score conveyor (prologue proj pp/pv
        # tiles ride in half-used conveyor slots) + 1 fin po + 1 P@V acc
        # (heads 0/1 accumulate in two sequential passes).
        ps_st = ctx.enter_context(tc.tile_pool(name="ps_st", bufs=3, space="PSUM"))
        ps_fin = ctx.enter_context(tc.tile_pool(name="ps_fin", bufs=1, space="PSUM"))
        ps_acc = ctx.enter_context(tc.tile_pool(name="ps_acc", bufs=1, space="PSUM"))

        def shtile(nm):
            return ps_st.tile([128, 2, 512], F32, tag="st", name=nm)[:, 0, :]

        def acctile(nm):
            return ps_acc.tile([128, 512], F32, tag="acc", name=nm)

        # --- load inputs ---------------------------------------------------
        # order: k01 weights + biases, then x first halves in 512-col
        # pieces (the first k-proj unit needs only the first piece of each
        # dcc), q01 weights interleaved, then the rest.
        for dcc in range(DC):
            nc.sync.dma_start(
                wqks[1][:, dcc * 128:(dcc + 1) * 128], wqk_d[1, dcc])
        nc.sync.dma_start(bqks[:], bqk_d[:])
        NPC = XH // 512  # 512-col pieces per half
        for pc in range(NPC):
            for dcc in range(DC):
                nc.sync.dma_start(
                    xts[dcc * (S // XH)][:, pc * 512:(pc + 1) * 512],
                    xT_d[dcc, :, pc * 512:(pc + 1) * 512])
            if pc == 0:
                for dcc in range(DC):
                    nc.sync.dma_start(
                        wqks[0][:, dcc * 128:(dcc + 1) * 128], wqk_d[0, dcc])
        for dcc in range(DC):
            nc.sync.dma_start(
                wqks[2][:, dcc * 128:(dcc + 1) * 128], wqk_d[2, dcc])
        for i in range(DC):
            for hh in range(1, S // XH):
                nc.sync.dma_start(xts[i * (S // XH) + hh][:],
                                  xT_d[i, :, hh * XH:(hh + 1) * XH])
        for i in range(DC):
            nc.sync.dma_start(wvs[i][:], wv_d[i])
        for i in range(2):
            nc.sync.dma_start(wos[i][:], wo_d[i])
        # dual-fp8 ldweights needs per-plane column count 64 or 128, so V
        # carries zero padding + a ones column (exp row-sum).  Heads 0/2 put
        # V in cols 0:64 + ones in col 64; head 1 puts ones in col 0 + V in
        # cols 64:128, so its P@V accumulator rows land at partitions 64:128
        # and heads 0+1 share one fin lhsT tile.
        nc.gpsimd.memset(warmt[:], 0.0)
        for h in (0, 2):
            nc.gpsimd.memset(v1s[h][:, :, 64:65], 1.0)
            nc.gpsimd.memset(v1s[h][:, :, 65:128], 0.0)
        nc.gpsimd.memset(v1s[1][:, :, 0:1], 1.0)
        nc.gpsimd.memset(v1s[1][:, :, 1:64], 0.0)
        for qc in range(NCH):
            nc.gpsimd.memset(at2[qc][HD:128, :], 0.0)

        # --- projections ---------------------------------------------------
        def proj_qk_unit(blk, sc):
            # blk0 = [q0 q1] -> qt01; blk1 = [k0 k1] -> kt01;
            # blk2 = [q2 k2]: q2 -> qt2d rows 0:64 (+DMA dup); k2 first-half
            # blocks staged + DMA partition-shifted to kt2d rows 0:64,
            # second half added directly to rows 64:128.
            pp = shtile(f"pp{blk}_{sc}")
            for dcc in range(DC):
                nc.tensor.matmul(
                    pp[:],
                    lhsT=wqks[blk][:, dcc * 128:(dcc + 1) * 128],
                    rhs=xth(dcc, sc * 512, 512),
                    start=(dcc == 0),
                    stop=(dcc == DC - 1),
                )
            sl = slice(sc * 512, (sc + 1) * 512)
            if blk == 0 or blk == 1:
                dst = qt01 if blk == 0 else kt01
                nc.vector.tensor_scalar_add(
                    dst[:, sl], pp[:], bqks[:, blk:blk + 1])
            else:
                nc.vector.tensor_scalar_add(
                    qt2d[0:64, sl], pp[0:64, :], bqks[0:64, 2:3])
                nc.sync.dma_start(qt2d[64:128, sl], qt2d[0:64, sl])
                if sc < 4:
                    k2s = small_pool.tile([128, 512], BF16, tag="k2s",
                                          name=f"k2s{sc}")
                    nc.vector.tensor_scalar_add(
                        k2s[64:128, :], pp[64:128, :], bqks[64:128, 2:3])
                    nc.sync.dma_start(kt2d[0:64, sl], k2s[64:128, :])
                else:
                    sl2 = slice((sc - 4) * 512, (sc - 3) * 512)
                    nc.vector.tensor_scalar_add(
                        kt2d[64:128, sl2], pp[64:128, :], bqks[64:128, 2:3])

        def proj_v(s0=0, s1=None):
            for sb in range(s0, SB if s1 is None else s1):
                pv = shtile(f"pv{sb}")
                pvv = pv[:, 0:NH_LOC * HD]
                for dcc in range(DC):
                    nc.tensor.matmul(
                        pvv,
                        lhsT=xth(dcc, sb * 128, 128),
                        rhs=wvs[dcc][:],
                        start=(dcc == 0),
                        stop=(dcc == DC - 1),
                    )
                nc.scalar.copy(v1s[0][:, sb, 0:64], pv[:, 0:HD])
                nc.scalar.copy(v1s[1][:, sb, 64:128], pv[:, HD:2 * HD])
                nc.scalar.copy(v1s[2][:, sb, 0:64], pv[:, 2 * HD:3 * HD])

        if level < 2:
            for blk in range(3):
                for sc in range(S // 512):
                    proj_qk_unit(blk, sc)
            proj_v()
            for sb in range(SB):
                ost = outst_pool.tile([128, D_MODEL], BF16, tag="ost",
                                      name=f"ost{sb}")
                nc.vector.memset(ost[:], 0.0)
                nc.sync.dma_start(out_d[sb * 128:(sb + 1) * 128, :], ost[:])
            return nc

        # --- attention -----------------------------------------------------
        ec = [0]  # exp engine round-robin counter

        def emit_exp(dst_ap, src_ap):
            c = ec[0]
            ec[0] += 1
            if (c % PAT_N) in DVE_PAT:
                nc.vector.tensor_scalar(
                    out=dst_ap.bitcast(I8),
                    in0=src_ap,
                    scalar1=SCHRAUD_A,
                    scalar2=SCHRAUD_B,
                    op0=mybir.AluOpType.mult,
                    op1=mybir.AluOpType.add,
                )
            else:
                nc.scalar.activation(dst_ap, src_ap, AF.Exp, scale=0.125)

        def scores_h01(qc, t0=0, t1=None, ptps=None):
            # ptp[t] holds blocks (2t, 2t+1) for heads 0+1 as
            # [128, pair-pos p, head e, 512].  Each st fill = 2 row-tiled
            # K=64 matmuls (h0 rows 0:64 -> plane 0, h1 rows 64:128 ->
            # plane 1, concurrent on the PE); one exp per st tile.
            if ptps is None:
                ptps = []
            qsl = slice(qc * CHUNK, (qc + 1) * CHUNK)
            for t in range(t0, KB // 2 if t1 is None else t1):
                ptp = pt_pool.tile([128, 2, 2, CHUNK], F8, tag="pt",
                                   name=f"pt01_{qc}_{t}")
                for p in (0, 1):
                    j = 2 * t + p
                    st = ps_st.tile([128, 2, CHUNK], F32, tag="st",
                                    name=f"st01_{qc}_{j}")
                    nc.tensor.matmul(
                        st[:, 0, :],
                        lhsT=kt01[0:64, j * 128:(j + 1) * 128],
                        rhs=qt01[0:64, qsl], start=True, stop=True)
                    nc.tensor.matmul(
                        st[:, 1, :],
                        lhsT=kt01[64:128, j * 128:(j + 1) * 128],
                        rhs=qt01[64:128, qsl], start=True, stop=True)
                    emit_exp(ptp[:, p, :, :], st[:, :, :])
                ptps.append(ptp)
            return ptps

        def scores_h2(qc, t0=0, t1=None, ptps=None):
            # ptp[t]: plane e=0 = blocks (2t, 2t+1), plane e=1 = blocks
            # (HKB+2t, HKB+2t+1); half e on partition rows 64e:64e+64.
            if ptps is None:
                ptps = []
            qsl = slice(qc * CHUNK, (qc + 1) * CHUNK)
            for t in range(t0, HKB // 2 if t1 is None else t1):
                ptp = pt_pool.tile([128, 2, 2, CHUNK], F8, tag="pt",
                                   name=f"pt2_{qc}_{t}")
                for p in (0, 1):
                    m = 2 * t + p
                    st = ps_st.tile([128, 2, CHUNK], F32, tag="st",
                                    name=f"st2_{qc}_{m}")
                    nc.tensor.matmul(
                        st[:, 0, :],
                        lhsT=kt2d[0:64, m * 128:(m + 1) * 128],
                        rhs=qt2d[0:64, qsl], start=True, stop=True)
                    nc.tensor.matmul(
                        st[:, 1, :],
                        lhsT=kt2d[64:128, m * 128:(m + 1) * 128],
                        rhs=qt2d[64:128, qsl], start=True, stop=True)
                    emit_exp(ptp[:, p, :, :], st[:, :, :])
                ptps.append(ptp)
            return ptps

        ones64 = ctile("ones64", [128, 64], BF16)
        nc.vector.memset(ones64[:], 1.0)

        def normalize(qc, h, acc):
            # reciprocal of the ones-column row, broadcast across partitions
            # via DRAM round trips (gpsimd DGE queue; off the critical path
            # by the fin delay); tmp in bf16.  For the last chunk, skip the
            # DRAM trips: reciprocal on the [1, 512] row directly and
            # broadcast with a K=1 matmul into a freed score-psum bank.
            sumrow, v0, v1_ = (0, 64, 128) if h == 1 else (64, 0, 64)
            dst = at2[qc] if h == 2 else atp[qc]
            tmp = small_pool.tile([128, CHUNK], BF16, tag="r1",
                                  name=f"r1_{h}_{qc}")
            nc.vector.tensor_copy(tmp[:], acc[:])
            if qc == NCH - 1:
                rq = rb_pool.tile([128, CHUNK], BF16, tag="rqf",
                                  name=f"rqf_{h}")
                sr = slice(sumrow, sumrow + 1)
                with nc.allow_low_precision(reason="softmax denom recip"):
                    nc.vector.reciprocal(rq[sr, :], tmp[sr, :])
                bc = ps_st.tile([128, 2, CHUNK], F32, tag="st",
                                name=f"bc_{h}")[:, 0, :]
                nc.tensor.matmul(bc[v0:v1_, :], lhsT=ones64[sr, :],
                                 rhs=rq[sr, :], start=True, stop=True)
                nc.vector.tensor_mul(
                    dst[v0:v1_, :], tmp[v0:v1_, :], bc[v0:v1_, :])
                return
            drs = dram_pool.tile([1, CHUNK], BF16, tag="drs",
                                 name=f"drs{h}_{qc}")
            nc.gpsimd.dma_start(drs[:], tmp[sumrow:sumrow + 1, :])
            rr4 = rb_pool.tile([128, CHUNK // 128], BF16, tag="rr4",
                               name=f"rr4_{h}_{qc}")
            nc.gpsimd.dma_start(rr4[:], drs[:])
            rq4 = rb_pool.tile([128, CHUNK // 128], F32, tag="rq4",
                               name=f"rq4_{h}_{qc}")
            nc.vector.reciprocal(rq4[:], rr4[:])
            dr2 = dram_pool.tile([1, CHUNK], F32, tag="dr2",
                                 name=f"dr2_{h}_{qc}")
            nc.gpsimd.dma_start(dr2[:], rq4[:])
            rbs = rb_pool.tile([128, CHUNK], F32, tag="rbs",
                               name=f"rbs{h}_{qc}")
            nc.gpsimd.dma_start(rbs[v0:v1_, :],
                                dr2[:].to_broadcast([HD, CHUNK]))
            nc.vector.tensor_mul(
                dst[v0:v1_, :],
                tmp[v0:v1_, :],
                rbs[v0:v1_, :],
            )

        def pav_pass(qc, e, ptps):
            # one head's P@V accumulation pass (all passes share one bank)
            acc = acctile(f"acc{e}_{qc}")
            n = KB // 2
            for t in range(n):
                nc.tensor.matmul(
                    acc[:],
                    lhsT=v1s[e][:, 2 * t:2 * t + 2, :],
                    rhs=ptps[t][:, :, e, :],
                    start=(t == 0),
                    stop=(t == n - 1),
                    perf_mode=DR,
                )
            normalize(qc, e, acc)

        def pav_h2(qc, ptps):
            acc = acctile(f"acc2_{qc}")
            n = HKB // 2
            for t in range(n):
                for e in (0, 1):
                    j0 = 2 * t + HKB * e
                    nc.tensor.matmul(
                        acc[:],
                        lhsT=v1s[2][:, j0:j0 + 2, :],
                        rhs=ptps[t][:, :, e, :],
                        start=(t == 0 and e == 0),
                        stop=(t == n - 1 and e == 1),
                        perf_mode=DR,
                    )
            normalize(qc, 2, acc)

        def fin(qc, sbs=None):
            # final projection for chunk qc's s-blocks: heads 0/1 share one
            # K=128 lhsT (atp); head 2 rides with K=128 zero padding (at2).
            # PSUM->SBUF copy on ACT (Copy shares the exp table set).
            if sbs is None:
                sbs = range(CHUNK // 128)
            for sb_in in sbs:
                sb = qc * (CHUNK // 128) + sb_in
                ost = outst_pool.tile([128, D_MODEL], BF16, tag="ost",
                                      name=f"ost{sb}")
                for (n0, n1) in ((0, 512), (512, D_MODEL)):
                    po = ps_fin.tile([128, 512], F32, tag="po",
                                     name=f"fp{sb}_{n0}")
                    pon = po[:, 0:n1 - n0]
                    sl = slice(sb_in * 128, (sb_in + 1) * 128)
                    nc.tensor.matmul(pon, lhsT=atp[qc][:, sl],
                                     rhs=wos[0][:, n0:n1],
                                     start=True, stop=False)
                    nc.tensor.matmul(pon, lhsT=at2[qc][:, sl],
                                     rhs=wos[1][:, n0:n1],
                                     start=False, stop=True)
                    nc.scalar.copy(ost[:, n0:n1], pon)
                nc.gpsimd.dma_start(out_d[sb * 128:(sb + 1) * 128, :], ost[:])

        # warm the PE HAM clock gate during the input DMA
        wps = ps_st.tile([128, 2, CHUNK], F32, tag="st", name="warm")
        for i in range(14):
            nc.tensor.matmul(wps[:, 0, :], lhsT=warmt[:, 0:128],
                             rhs=warmt[:, 128:128 + CHUNK],
                             start=(i == 0), stop=(i == 13))
        # prologue: score fills interleave with EVERY projection phase so
        # the 3-deep score-PSUM conveyor (and thus the exp engines) never
        # runs dry: k-proj with (h01, qc0) pairs, q-proj with (h01, qc1)
        # pairs, head-2 proj (first/second-half sc paired) with (h2, qc0),
        # and proj_v feeds ACT via its V-quantize copies.
        NSC = S // 512
        proj_qk_unit(1, 0)
        proj_qk_unit(0, 0)
        pts_u0 = []
        scores_h01(0, 0, 2, pts_u0)
        for sc in range(1, NSC):
            proj_qk_unit(1, sc)
            scores_h01(0, 2 * sc, 2 * sc + 2, pts_u0)
        for sc in range(1, NSC):
            proj_qk_unit(0, sc)
        for sc in range(NSC):
            proj_qk_unit(2, sc)
        pts_u1 = scores_h2(0)
        pts_u2 = scores_h01(1, 0, KB // 4)  # first half; rest at u=0
        proj_v()

        # main loop over pair-units u = 2*qc + kind (kind 0 = heads 0+1,
        # kind 1 = head 2).  Each normalize is emitted right after its P@V
        # pass and BEFORE the next score batch, so its PSUM-freeing copy
        # sits ahead of the exp calls in the engine queues (the shared acc
        # bank otherwise stalls the next pass).
        NU = 2 * NCH
        ptss = {0: pts_u0, 1: pts_u1, 2: pts_u2}
        for u in range(NU):
            qc, kind = divmod(u, 2)
            if level >= 3 and kind == 0 and qc > 0:
                fin(qc - 1, sbs=(2, 3))
            ptps = ptss.pop(u)
            if kind == 0:
                pav_pass(qc, 0, ptps)
                if u == 0:
                    scores_h01(1, KB // 4, None, ptss[2])
                if u + 3 < NU:
                    ptss[u + 3] = scores_h2(qc + 1)
                pav_pass(qc, 1, ptps)
            else:
                pav_h2(qc, ptps)
                if u + 3 < NU:
                    ptss[u + 3] = scores_h01(qc + 2)
                if level >= 3:
                    fin(qc, sbs=(0, 1))

        if level < 3:
            for sb in range(SB):
                ost = outst_pool.tile([128, D_MODEL], BF16, tag="ost",
                                      name=f"ost{sb}")
                nc.vector.memset(ost[:], 0.0)
                nc.sync.dma_start(out_d[sb * 128:(sb + 1) * 128, :], ost[:])
            return nc
        fin(NCH - 1, sbs=(2, 3))

    return nc


def make_nc(S=4096, level=3):
    nc = bacc.Bacc(None, target_bir_lowering=False, debug=False)
    build(nc, S, level=level)
    nc.compile()
    return nc


def shard_inputs(x, Wq, bq, Wk, bk, Wv, bv, Wo, bo, S):
    """Host-side packing of the 8 per-core input maps (bf16 casts included)."""
    import ml_dtypes

    bf = ml_dtypes.bfloat16
    in_maps = []
    for c in range(N_CORES):
        b = c // 4
        h0 = NH_LOC * (c % 4)
        cs, ce = h0 * HD, (h0 + NH_LOC) * HD
        xT = np.ascontiguousarray(x[b].T).astype(bf).reshape(DC, 128, S)

        def blkify(w2):  # [768, 128] -> [DC, 128, 128]
            return np.ascontiguousarray(w2).astype(bf).reshape(DC, 128, 128)

        wqk = np.stack([
            blkify(Wq[:, cs:cs + 2 * HD]),
            blkify(Wk[:, cs:cs + 2 * HD]),
            blkify(np.concatenate([Wq[:, cs + 2 * HD:ce],
                                   Wk[:, cs + 2 * HD:ce]], axis=1)),
        ])
        bqk = np.stack([
            bq[cs:cs + 2 * HD],
            bk[cs:cs + 2 * HD],
            np.concatenate([bq[cs + 2 * HD:ce], bk[cs + 2 * HD:ce]]),
        ], axis=1).astype(np.float32)  # [128, 3]
        wv = np.ascontiguousarray(Wv[:, cs:ce]).astype(bf).reshape(
            DC, 128, NH_LOC * HD)
        wo = np.zeros((2, 128, D_MODEL), np.float32)
        wo[0, 0:HD, :] = Wo[cs:cs + HD, :]
        wo[0, HD:128, :] = Wo[cs + HD:cs + 2 * HD, :]
        wo[1, 0:HD, :] = Wo[cs + 2 * HD:ce, :]
        wo = wo.astype(bf)
        in_maps.append({"xT": xT, "wqk": wqk, "bqk": bqk, "wv": wv, "wo": wo})
    return in_maps


_NC_CACHE = {}


def kernel(x, Wq, bq, Wk, bk, Wv, bv, Wo, bo):
    from concourse import bass_utils

    x = np.asarray(x, np.float32)
    Wq, bq = np.asarray(Wq, np.float32), np.asarray(bq, np.float32)
    Wk, bk = np.asarray(Wk, np.float32), np.asarray(bk, np.float32)
    Wv, bv = np.asarray(Wv, np.float32), np.asarray(bv, np.float32)
    Wo, bo = np.asarray(Wo, np.float32), np.asarray(bo, np.float32)
    B, S, D = x.shape
    assert (B, D) == (2, D_MODEL)
    if S not in _NC_CACHE:
        _NC_CACHE[S] = make_nc(S)
    nc = _NC_CACHE[S]

    in_maps = shard_inputs(x, Wq, bq, Wk, bk, Wv, bv, Wo, bo, S)
    res = bass_utils.run_bass_kernel_spmd(nc, in_maps, core_ids=list(range(N_CORES)))

    # host reduction: sum head-group partials per batch, add bias terms
    bias = (bo.astype(np.float32)
            + bv.astype(np.float32) @ Wo.astype(np.float32))  # [768]
    out = np.empty((B, S, D_MODEL), np.float32)
    for b in range(B):
        acc = res.results[4 * b]["out"].astype(np.float32)
        for c in range(4 * b + 1, 4 * b + 4):
            acc = acc + res.results[c]["out"].astype(np.float32)
        out[b] = acc + bias
    return out


# revision 28
# speedup vs baseline: 1.0607x; 1.0607x over previous
"""Multi-head attention (B=2, S=4096, D=768, H=12, hd=64) on 8 trn2 NeuronCores.

Sharding: core c -> batch b = c//4, heads [3*(c%4), 3*(c%4)+3)  (batch- and
head-parallel; no device collectives).  Each core computes the partial
output  sum_h softmax((x Wq_h + bq_h)(x Wk_h + bk_h)^T / 8) (x Wv_h) Wo_h
for its 3 heads as a full [S, 768] bf16 tensor; the host sums the 4 partials
per batch and adds the bias terms (bo + bv @ Wo, since softmax rows sum to 1).

Key device-side structure (v2 of this kernel; baseline was K=128
zero-padded scores with all-ACT exp at 467us):
  - scores run as ROW-TILED K=64 matmul pairs: heads 0/1 live on SBUF
    partition halves 0:64 / 64:128 of merged qt01/kt01 tiles, so two score
    matmuls (tile_position (0,0)/(64,0), auto-derived from AP base
    partitions) stream CONCURRENTLY through the PE array (trace-verified
    dstart ~3ns, pair completes in one matmul's duration) — 2x score
    throughput vs the K=128 zero-padded form.  Head 2 pairs with itself:
    k-blocks 0:16 on rows 0:64, 16:32 on rows 64:128 (q2 duplicated onto
    both halves, k2 first half partition-shifted, via SBUF-SBUF DMA).
  - exp split 6/11 ACT (true exp -> fp8e4m3), 5/11 DVE via a Schraudolph
    bit-trick: int8(S/ln2 + 56) IS the fp8e4m3 bit pattern of exp(S/8)
    (rel_l2 9.9e-3 vs 8.4e-3 all-ACT, sim-verified; constant insensitive
    +-0.5 and rounding-mode independent since the softmax denominator is
    computed from the same quantized P).  The steady state runs both
    engines ~90% busy — exp throughput is the kernel's pacing resource.
  - P@V via fp8 DoubleRow; P tiles [128, 2, 2, 512] (pair-pos x head x q):
    one exp fills both heads' planes of one pair-position, each DR call
    reads one head's plane pair at stride 1024B.  Heads 0/1 accumulate in
    two sequential passes sharing one PSUM bank; each pass's accumulator
    is normalized (freed) before the next score batch is emitted so the
    bank's WAR chain stays ahead of the exp queues.
  - prologue: fine-grained x DMA + k-proj interleaved with the first
    q-chunk's scores so the exp stream starts ~22us in (vs 54us); one
    extra half-unit of qc1 scores hoisted.  fc_out copies and V-quantize
    on ACT (Copy/Identity share the exp act-table set); bf16 output
    (host sums the 4 partials per batch in f32).
  - normalize: ones-column row sum -> reciprocal via [128,4] DRAM
    round-trip relayout (gpsimd DGE queue, hidden by the one-chunk fin
    lag); the last chunk skips DRAM and broadcasts with a K=1 matmul.
  - PSUM banks: 3x[128,2,512] score conveyor (6; prologue proj tiles ride
    in half-used conveyor slots) + 1 fin + 1 P@V accumulator = 8.
"""

import numpy as np
from contextlib import ExitStack

import concourse.bass as bass
import concourse.bacc as bacc
import concourse.mybir as mybir
from concourse import tile

BF16 = mybir.dt.bfloat16
F32 = mybir.dt.float32
F8 = mybir.dt.float8e4
I8 = mybir.dt.int8
AF = mybir.ActivationFunctionType
DR = mybir.MatmulPerfMode.DoubleRow

D_MODEL = 768
N_HEADS = 12
HD = 64
N_CORES = 8
NH_LOC = 3           # heads per core
DC = D_MODEL // 128  # 6 chunks of d_model
CHUNK = 512          # q columns processed per score chunk

# Schraudolph int8 exp: int8(round(S * 1/ln2 + 56)) == fp8e4m3 bits of
# exp(S/8) (3 mantissa bits, bias 7; valid for |S| < ~49).
SCHRAUD_A = float(1.0 / np.log(2.0))
SCHRAUD_B = 56.0
# exp-call engine assignment: DVE takes these slots of every PAT_N calls
DVE_PAT = (1, 3, 5, 7, 9)
PAT_N = 11


def build(nc, S, level=3):
    SB = S // 128     # seq blocks of 128
    NCH = S // CHUNK  # q chunks
    KB = S // 128     # k blocks of 128
    HKB = KB // 2     # k blocks per partition half (head 2)

    xT_d = nc.declare_dram_parameter("xT", [DC, 128, S], BF16, isOutput=False)
    wqk_d = nc.declare_dram_parameter("wqk", [3, DC, 128, 128], BF16, isOutput=False)
    bqk_d = nc.declare_dram_parameter("bqk", [128, 3], F32, isOutput=False)
    wv_d = nc.declare_dram_parameter("wv", [DC, 128, NH_LOC * HD], BF16, isOutput=False)
    wo_d = nc.declare_dram_parameter("wo", [2, 128, D_MODEL], BF16, isOutput=False)
    out_d = nc.declare_dram_parameter("out", [S, D_MODEL], BF16, isOutput=True)

    with tile.TileContext(nc) as tc, ExitStack() as ctx:
        const = ctx.enter_context(tc.tile_pool(name="const", bufs=1))

        def ctile(name, shape, dt):
            return const.tile(shape, dt, tag=name, name=name)

        # --- constants / long-lived tensors -------------------------------
        XH = S // 2 if S >= 1024 else S   # xT column-half size
        xts = [ctile(f"xt{i}", [128, XH], BF16)
               for i in range(DC * (S // XH))]

        def xth(dcc, off, ln):
            t = xts[dcc * (S // XH) + off // XH]
            lo = off % XH
            return t[:, lo:lo + ln]
        wqks = [ctile(f"wqk{i}", [128, DC * 128], BF16) for i in range(3)]
        bqks = ctile("bqk", [128, 3], F32)
        wvs = [ctile(f"wv{i}", [128, NH_LOC * HD], BF16) for i in range(DC)]
        wos = [ctile(f"wo{i}", [128, D_MODEL], BF16) for i in range(2)]
        v1s = [ctile(f"v1_{h}", [128, KB, 128], F8) for h in range(NH_LOC)]
        warmt = ctile("warm", [128, 640], BF16)
        # merged q/k tiles: head 0 on rows 0:64, head 1 on rows 64:128
        qt01 = ctile("qt01", [128, S], BF16)
        kt01 = ctile("kt01", [128, S], BF16)
        # head 2: q duplicated on both halves; k blocks 0:HKB on rows 0:64,
        # blocks HKB:KB on rows 64:128 (block m at cols (m%HKB)*128)
        qt2d = ctile("qt2d", [128, S], BF16)
        kt2d = ctile("kt2d", [128, HKB * 128], BF16)
        # fin lhsT: heads 0 (rows 0:64) + 1 (rows 64:128) share atp;
        # at2: head 2 rows 0:64, rows 64:128 zero-padded
        atp = [ctile(f"atp{qc}", [128, CHUNK], BF16) for qc in range(NCH)]
        at2 = [ctile(f"at2_{qc}", [128, CHUNK], BF16) for qc in range(NCH)]

        pt_pool = ctx.enter_context(tc.tile_pool(name="pt", bufs=38))
        outst_pool = ctx.enter_context(tc.tile_pool(name="outst", bufs=2))
        small_pool = ctx.enter_context(tc.tile_pool(name="small", bufs=2))
        rb_pool = ctx.enter_context(tc.tile_pool(name="rb", bufs=2))
        dram_pool = ctx.enter_context(tc.tile_pool(name="drs", bufs=3, space="DRAM"))
        # PSUM (8 banks): 3x[128,2,512] score conveyor (prologue proj pp/pv
        # tiles ride in half-used conveyor slots) + 1 fin po + 1 P@V acc
        # (heads 0/1 accumulate in two sequential passes).
        ps_st = ctx.enter_context(tc.tile_pool(name="ps_st", bufs=3, space="PSUM"))
        ps_fin = ctx.enter_context(tc.tile_pool(name="ps_fin", bufs=1, space="PSUM"))
        ps_acc = ctx.enter_context(tc.tile_pool(name="ps_acc", bufs=1, space="PSUM"))

        def shtile(nm):
            return ps_st.tile([128, 2, 512], F32, tag="st", name=nm)[:, 0, :]

        def acctile(nm):
            return ps_acc.tile([128, 512], F32, tag="acc", name=nm)

        # --- load inputs ---------------------------------------------------
        # order: k01 weights + biases, then x first halves in 512-col
        # pieces (the first k-proj unit needs only the first piece of each
        # dcc), q01 weights interleaved, then the rest.
        for dcc in range(DC):
            nc.sync.dma_start(
                wqks[1][:, dcc * 128:(dcc + 1) * 128], wqk_d[1, dcc])
        nc.sync.dma_start(bqks[:], bqk_d[:])
        NPC = XH // 512  # 512-col pieces per half
        for pc in range(NPC):
            for dcc in range(DC):
                nc.sync.dma_start(
                    xts[dcc * (S // XH)][:, pc * 512:(pc + 1) * 512],
                    xT_d[dcc, :, pc * 512:(pc + 1) * 512])
            if pc == 0:
                for dcc in range(DC):
                    nc.sync.dma_start(
                        wqks[0][:, dcc * 128:(dcc + 1) * 128], wqk_d[0, dcc])
        for dcc in range(DC):
            nc.sync.dma_start(
                wqks[2][:, dcc * 128:(dcc + 1) * 128], wqk_d[2, dcc])
        for i in range(DC):
            for hh in range(1, S // XH):
                nc.sync.dma_start(xts[i * (S // XH) + hh][:],
                                  xT_d[i, :, hh * XH:(hh + 1) * XH])
        for i in range(DC):
            nc.sync.dma_start(wvs[i][:], wv_d[i])
        for i in range(2):
            nc.sync.dma_start(wos[i][:], wo_d[i])
        # dual-fp8 ldweights needs per-plane column count 64 or 128, so V
        # carries zero padding + a ones column (exp row-sum).  Heads 0/2 put
        # V in cols 0:64 + ones in col 64; head 1 puts ones in col 0 + V in
        # cols 64:128, so its P@V accumulator rows land at partitions 64:128
        # and heads 0+1 share one fin lhsT tile.
        nc.gpsimd.memset(warmt[:], 0.0)
        for h in (0, 2):
            nc.gpsimd.memset(v1s[h][:, :, 64:65], 1.0)
            nc.gpsimd.memset(v1s[h][:, :, 65:128], 0.0)
        nc.gpsimd.memset(v1s[1][:, :, 0:1], 1.0)
        nc.gpsimd.memset(v1s[1][:, :, 1:64], 0.0)
        for qc in range(NCH):
            nc.gpsimd.memset(at2[qc][HD:128, :], 0.0)

        # --- projections ---------------------------------------------------
        def proj_qk_unit(blk, sc):
            # blk0 = [q0 q1] -> qt01; blk1 = [k0 k1] -> kt01;
            # blk2 = [q2 k2]: q2 -> qt2d rows 0:64 (+DMA dup); k2 first-half
            # blocks staged + DMA partition-shifted to kt2d rows 0:64,
            # second half added directly to rows 64:128.
            pp = shtile(f"pp{blk}_{sc}")
            for dcc in range(DC):
                nc.tensor.matmul(
                    pp[:],
                    lhsT=wqks[blk][:, dcc * 128:(dcc + 1) * 128],
                    rhs=xth(dcc, sc * 512, 512),
                    start=(dcc == 0),
                    stop=(dcc == DC - 1),
                )
            sl = slice(sc * 512, (sc + 1) * 512)
            if blk == 0 or blk == 1:
                dst = qt01 if blk == 0 else kt01
                nc.vector.tensor_scalar_add(
                    dst[:, sl], pp[:], bqks[:, blk:blk + 1])
            else:
                nc.vector.tensor_scalar_add(
                    qt2d[0:64, sl], pp[0:64, :], bqks[0:64, 2:3])
                nc.sync.dma_start(qt2d[64:128, sl], qt2d[0:64, sl])
                if sc < 4:
                    k2s = small_pool.tile([128, 512], BF16, tag="k2s",
                                          name=f"k2s{sc}")
                    nc.vector.tensor_scalar_add(
                        k2s[64:128, :], pp[64:128, :], bqks[64:128, 2:3])
                    nc.sync.dma_start(kt2d[0:64, sl], k2s[64:128, :])
                else:
                    sl2 = slice((sc - 4) * 512, (sc - 3) * 512)
                    nc.vector.tensor_scalar_add(
                        kt2d[64:128, sl2], pp[64:128, :], bqks[64:128, 2:3])

        def proj_v(s0=0, s1=None):
            for sb in range(s0, SB if s1 is None else s1):
                pv = shtile(f"pv{sb}")
                pvv = pv[:, 0:NH_LOC * HD]
                for dcc in range(DC):
                    nc.tensor.matmul(
                        pvv,
                        lhsT=xth(dcc, sb * 128, 128),
                        rhs=wvs[dcc][:],
                        start=(dcc == 0),
                        stop=(dcc == DC - 1),
                    )
                nc.scalar.copy(v1s[0][:, sb, 0:64], pv[:, 0:HD])
                nc.scalar.copy(v1s[1][:, sb, 64:128], pv[:, HD:2 * HD])
                nc.scalar.copy(v1s[2][:, sb, 0:64], pv[:, 2 * HD:3 * HD])

        if level < 2:
            for blk in range(3):
                for sc in range(S // 512):
                    proj_qk_unit(blk, sc)
            proj_v()
            for sb in range(SB):
                ost = outst_pool.tile([128, D_MODEL], BF16, tag="ost",
                                      name=f"ost{sb}")
                nc.vector.memset(ost[:], 0.0)
                nc.sync.dma_start(out_d[sb * 128:(sb + 1) * 128, :], ost[:])
            return nc

        # --- attention -----------------------------------------------------
        ec = [0]  # exp engine round-robin counter

        def emit_exp(dst_ap, src_ap):
            c = ec[0]
            ec[0] += 1
            if (c % PAT_N) in DVE_PAT:
                nc.vector.tensor_scalar(
                    out=dst_ap.bitcast(I8),
                    in0=src_ap,
                    scalar1=SCHRAUD_A,
                    scalar2=SCHRAUD_B,
                    op0=mybir.AluOpType.mult,
                    op1=mybir.AluOpType.add,
                )
            else:
                nc.scalar.activation(dst_ap, src_ap, AF.Exp, scale=0.125)

        def scores_h01(qc, t0=0, t1=None, ptps=None):
            # ptp[t] holds blocks (2t, 2t+1) for heads 0+1 as
            # [128, pair-pos p, head e, 512].  Each st fill = 2 row-tiled
            # K=64 matmuls (h0 rows 0:64 -> plane 0, h1 rows 64:128 ->
            # plane 1, concurrent on the PE); one exp per st tile.
            if ptps is None:
                ptps = []
            qsl = slice(qc * CHUNK, (qc + 1) * CHUNK)
            for t in range(t0, KB // 2 if t1 is None else t1):
                ptp = pt_pool.tile([128, 2, 2, CHUNK], F8, tag="pt",
                                   name=f"pt01_{qc}_{t}")
                for p in (0, 1):
                    j = 2 * t + p
                    st = ps_st.tile([128, 2, CHUNK], F32, tag="st",
                                    name=f"st01_{qc}_{j}")
                    nc.tensor.matmul(
                        st[:, 0, :],
                        lhsT=kt01[0:64, j * 128:(j + 1) * 128],
                        rhs=qt01[0:64, qsl], start=True, stop=True)
                    nc.tensor.matmul(
                        st[:, 1, :],
                        lhsT=kt01[64:128, j * 128:(j + 1) * 128],
                        rhs=qt01[64:128, qsl], start=True, stop=True)
                    emit_exp(ptp[:, p, :, :], st[:, :, :])
                ptps.append(ptp)
            return ptps

        def scores_h2(qc, t0=0, t1=None, ptps=None):
            # ptp[t]: plane e=0 = blocks (2t, 2t+1), plane e=1 = blocks
            # (HKB+2t, HKB+2t+1); half e on partition rows 64e:64e+64.
            if ptps is None:
                ptps = []
            qsl = slice(qc * CHUNK, (qc + 1) * CHUNK)
            for t in range(t0, HKB // 2 if t1 is None else t1):
                ptp = pt_pool.tile([128, 2, 2, CHUNK], F8, tag="pt",
                                   name=f"pt2_{qc}_{t}")
                for p in (0, 1):
                    m = 2 * t + p
                    st = ps_st.tile([128, 2, CHUNK], F32, tag="st",
                                    name=f"st2_{qc}_{m}")
                    nc.tensor.matmul(
                        st[:, 0, :],
                        lhsT=kt2d[0:64, m * 128:(m + 1) * 128],
                        rhs=qt2d[0:64, qsl], start=True, stop=True)
                    nc.tensor.matmul(
                        st[:, 1, :],
                        lhsT=kt2d[64:128, m * 128:(m + 1) * 128],
                        rhs=qt2d[64:128, qsl], start=True, stop=True)
                    emit_exp(ptp[:, p, :, :], st[:, :, :])
                ptps.append(ptp)
            return ptps

        ones64 = ctile("ones64", [128, 64], BF16)
        nc.vector.memset(ones64[:], 1.0)

        def normalize(qc, h, acc):
            # reciprocal of the ones-column row, broadcast across partitions
            # via DRAM round trips (gpsimd DGE queue; off the critical path
            # by the fin delay); tmp in bf16.  For the last chunk, skip the
            # DRAM trips: reciprocal on the [1, 512] row directly and
            # broadcast with a K=1 matmul into a freed score-psum bank.
            sumrow, v0, v1_ = (0, 64, 128) if h == 1 else (64, 0, 64)
            dst = at2[qc] if h == 2 else atp[qc]
            tmp = small_pool.tile([128, CHUNK], BF16, tag="r1",
                                  name=f"r1_{h}_{qc}")
            nc.vector.tensor_copy(tmp[:], acc[:])
            if qc == NCH - 1:
                rq = rb_pool.tile([128, CHUNK], BF16, tag="rqf",
                                  name=f"rqf_{h}")
                sr = slice(sumrow, sumrow + 1)
                with nc.allow_low_precision(reason="softmax denom recip"):
                    nc.vector.reciprocal(rq[sr, :], tmp[sr, :])
                bc = ps_st.tile([128, 2, CHUNK], F32, tag="st",
                                name=f"bc_{h}")[:, 0, :]
                nc.tensor.matmul(bc[v0:v1_, :], lhsT=ones64[sr, :],
                                 rhs=rq[sr, :], start=True, stop=True)
                nc.vector.tensor_mul(
                    dst[v0:v1_, :], tmp[v0:v1_, :], bc[v0:v1_, :])
                return
            drs = dram_pool.tile([1, CHUNK], BF16, tag="drs",
                                 name=f"drs{h}_{qc}")
            nc.gpsimd.dma_start(drs[:], tmp[sumrow:sumrow + 1, :])
            rr4 = rb_pool.tile([128, CHUNK // 128], BF16, tag="rr4",
                               name=f"rr4_{h}_{qc}")
            nc.gpsimd.dma_start(rr4[:], drs[:])
            rq4 = rb_pool.tile([128, CHUNK // 128], F32, tag="rq4",
                               name=f"rq4_{h}_{qc}")
            nc.vector.reciprocal(rq4[:], rr4[:])
            dr2 = dram_pool.tile([1, CHUNK], F32, tag="dr2",
                                 name=f"dr2_{h}_{qc}")
            nc.gpsimd.dma_start(dr2[:], rq4[:])
            rbs = rb_pool.tile([128, CHUNK], F32, tag="rbs",
                               name=f"rbs{h}_{qc}")
            nc.gpsimd.dma_start(rbs[v0:v1_, :],
                                dr2[:].to_broadcast([HD, CHUNK]))
            nc.vector.tensor_mul(
                dst[v0:v1_, :],
                tmp[v0:v1_, :],
                rbs[v0:v1_, :],
            )

        def pav_pass(qc, e, ptps):
            # one head's P@V accumulation pass (all passes share one bank)
            acc = acctile(f"acc{e}_{qc}")
            n = KB // 2
            for t in range(n):
                nc.tensor.matmul(
                    acc[:],
                    lhsT=v1s[e][:, 2 * t:2 * t + 2, :],
                    rhs=ptps[t][:, :, e, :],
                    start=(t == 0),
                    stop=(t == n - 1),
                    perf_mode=DR,
                )
            normalize(qc, e, acc)

        def pav_h2(qc, ptps):
            acc = acctile(f"acc2_{qc}")
            n = HKB // 2
            for t in range(n):
                for e in (0, 1):
                    j0 = 2 * t + HKB * e
                    nc.tensor.matmul(
                        acc[:],
                        lhsT=v1s[2][:, j0:j0 + 2, :],
                        rhs=ptps[t][:, :, e, :],
                        start=(t == 0 and e == 0),
                        stop=(t == n - 1 and e == 1),
                        perf_mode=DR,
                    )
            normalize(qc, 2, acc)

        def fin(qc, sbs=None):
            # final projection for chunk qc's s-blocks: heads 0/1 share one
            # K=128 lhsT (atp); head 2 rides with K=128 zero padding (at2).
            # PSUM->SBUF copy on ACT (Copy shares the exp table set).
            if sbs is None:
                sbs = range(CHUNK // 128)
            for sb_in in sbs:
                sb = qc * (CHUNK // 128) + sb_in
                ost = outst_pool.tile([128, D_MODEL], BF16, tag="ost",
                                      name=f"ost{sb}")
                for (n0, n1) in ((0, 512), (512, D_MODEL)):
                    po = ps_fin.tile([128, 512], F32, tag="po",
                                     name=f"fp{sb}_{n0}")
                    pon = po[:, 0:n1 - n0]
                    sl = slice(sb_in * 128, (sb_in + 1) * 128)
                    nc.tensor.matmul(pon, lhsT=atp[qc][:, sl],
                                     rhs=wos[0][:, n0:n1],
                                     start=True, stop=False)
                    nc.tensor.matmul(pon, lhsT=at2[qc][:, sl],
                                     rhs=wos[1][:, n0:n1],
                                     start=False, stop=True)
                    nc.scalar.copy(ost[:, n0:n1], pon)
                nc.gpsimd.dma_start(out_d[sb * 128:(sb + 1) * 128, :], ost[:])

        # warm the PE HAM clock gate during the input DMA
        wps = ps_st.tile([128, 2, CHUNK], F32, tag="st", name="warm")
        for i in range(14):
            nc.tensor.matmul(wps[:, 0, :], lhsT=warmt[:, 0:128],
                             rhs=warmt[:, 128:128 + CHUNK],
                             start=(i == 0), stop=(i == 13))
        # prologue: score fills interleave with EVERY projection phase so
        # the 3-deep score-PSUM conveyor (and thus the exp engines) never
        # runs dry: k-proj with (h01, qc0) pairs, q-proj with (h01, qc1)
        # pairs, head-2 proj (first/second-half sc paired) with (h2, qc0),
        # and proj_v feeds ACT via its V-quantize copies.
        NSC = S // 512
        proj_qk_unit(1, 0)
        proj_qk_unit(0, 0)
        pts_u0 = []
        scores_h01(0, 0, 2, pts_u0)
        for sc in range(1, NSC):
            proj_qk_unit(1, sc)
            scores_h01(0, 2 * sc, 2 * sc + 2, pts_u0)
        for sc in range(1, NSC):
            proj_qk_unit(0, sc)
        for sc in range(NSC):
            proj_qk_unit(2, sc)
        pts_u1 = scores_h2(0)
        pts_u2 = scores_h01(1, 0, KB // 4)  # first half; rest at u=0
        proj_v()

        # main loop over pair-units u = 2*qc + kind (kind 0 = heads 0+1,
        # kind 1 = head 2).  Each normalize is emitted right after its P@V
        # pass and BEFORE the next score batch, so its PSUM-freeing copy
        # sits ahead of the exp calls in the engine queues (the shared acc
        # bank otherwise stalls the next pass).
        NU = 2 * NCH
        ptss = {0: pts_u0, 1: pts_u1, 2: pts_u2}
        for u in range(NU):
            qc, kind = divmod(u, 2)
            if level >= 3 and kind == 0 and qc > 0:
                fin(qc - 1, sbs=(2, 3))
            ptps = ptss.pop(u)
            if kind == 0:
                pav_pass(qc, 0, ptps)
                if u == 0:
                    scores_h01(1, KB // 4, None, ptss[2])
                if u + 3 < NU:
                    ptss[u + 3] = scores_h2(qc + 1)
                pav_pass(qc, 1, ptps)
            else:
                pav_h2(qc, ptps)
                if u + 3 < NU:
                    ptss[u + 3] = scores_h01(qc + 2)
                if level >= 3:
                    fin(qc, sbs=(0, 1))

        if level < 3:
            for sb in range(SB):
                ost = outst_pool.tile([128, D_MODEL], BF16, tag="ost",
                                      name=f"ost{sb}")
                nc.vector.memset(ost[:], 0.0)
                nc.sync.dma_start(out_d[sb * 128:(sb + 1) * 128, :], ost[:])
            return nc
        fin(NCH - 1, sbs=(2, 3))

    return nc


def make_nc(S=4096, level=3):
    nc = bacc.Bacc(None, target_bir_lowering=False, debug=False)
    build(nc, S, level=level)
    nc.compile()
    return nc


def shard_inputs(x, Wq, bq, Wk, bk, Wv, bv, Wo, bo, S):
    """Host-side packing of the 8 per-core input maps (bf16 casts included)."""
    import ml_dtypes

    bf = ml_dtypes.bfloat16
    in_maps = []
    for c in range(N_CORES):
        b = c // 4
        h0 = NH_LOC * (c % 4)
        cs, ce = h0 * HD, (h0 + NH_LOC) * HD
        xT = np.ascontiguousarray(x[b].T).astype(bf).reshape(DC, 128, S)

        def blkify(w2):  # [768, 128] -> [DC, 128, 128]
            return np.ascontiguousarray(w2).astype(bf).reshape(DC, 128, 128)

        wqk = np.stack([
            blkify(Wq[:, cs:cs + 2 * HD]),
            blkify(Wk[:, cs:cs + 2 * HD]),
            blkify(np.concatenate([Wq[:, cs + 2 * HD:ce],
                                   Wk[:, cs + 2 * HD:ce]], axis=1)),
        ])
        bqk = np.stack([
            bq[cs:cs + 2 * HD],
            bk[cs:cs + 2 * HD],
            np.concatenate([bq[cs + 2 * HD:ce], bk[cs + 2 * HD:ce]]),
        ], axis=1).astype(np.float32)  # [128, 3]
        wv = np.ascontiguousarray(Wv[:, cs:ce]).astype(bf).reshape(
            DC, 128, NH_LOC * HD)
        wo = np.zeros((2, 128, D_MODEL), np.float32)
        wo[0, 0:HD, :] = Wo[cs:cs + HD, :]
        wo[0, HD:128, :] = Wo[cs + HD:cs + 2 * HD, :]
        wo[1, 0:HD, :] = Wo[cs + 2 * HD:ce, :]
        wo = wo.astype(bf)
        in_maps.append({"xT": xT, "wqk": wqk, "bqk": bqk, "wv": wv, "wo": wo})
    return in_maps


_NC_CACHE = {}


def kernel(x, Wq, bq, Wk, bk, Wv, bv, Wo, bo):
    from concourse import bass_utils

    x = np.asarray(x, np.float32)
    Wq, bq = np.asarray(Wq, np.float32), np.asarray(bq, np.float32)
    Wk, bk = np.asarray(Wk, np.float32), np.asarray(bk, np.float32)
    Wv, bv = np.asarray(Wv, np.float32), np.asarray(bv, np.float32)
    Wo, bo = np.asarray(Wo, np.float32), np.asarray(bo, np.float32)
    B, S, D = x.shape
    assert (B, D) == (2, D_MODEL)
    if S not in _NC_CACHE:
        _NC_CACHE[S] = make_nc(S)
    nc = _NC_CACHE[S]

    in_maps = shard_inputs(x, Wq, bq, Wk, bk, Wv, bv, Wo, bo, S)
    res = bass_utils.run_bass_kernel_spmd(nc, in_maps, core_ids=list(range(N_CORES)))

    # host reduction: sum head-group partials per batch, add bias terms
    bias = (bo.astype(np.float32)
            + bv.astype(np.float32) @ Wo.astype(np.float32))  # [768]
    out = np.empty((B, S, D_MODEL), np.float32)
    for b in range(B):
        acc = res.results[4 * b]["out"].astype(np.float32)
        for c in range(4 * b + 1, 4 * b + 4):
            acc = acc + res.results[c]["out"].astype(np.float32)
        out[b] = acc + bias
    return out
